# revision 1
# baseline (speedup 1.0000x reference)
"""Trainium2 Bass kernel: 4096x4096 fp32 image, 7x7 valid cross-correlation + bias.

Strategy
--------
Column-shard the image across 8 NeuronCores: core m computes output columns
[512*m, 512*m+512) (core 7 padded; image columns padded to 4102 on host, so
every core sees an identical input shard = 512 columns + 6 halo columns).

On each core the conv runs on the tensor engine as banded-Toeplitz matmuls:
an output row band of M=122 rows uses K=128 input rows (M + kh-1) as the
moving operand and contracts against seven stationary matrices A_dj[128, 128],
A_dj[k, m] = w[k-m, dj] (zero outside the band / beyond column 121).  The
seven column taps dj become free-axis shifts of the moving operand
(rhs = x[:, dj:dj+512]) accumulated in one PSUM bank via start/stop.

Layout: the host prepacks each shard band-partition-major, xs[p, b, c] =
x[122*b + p, c], so one DMA of G=8 bands reads a contiguous 8.3 KB run per
partition (128 descriptors/transfer instead of 1024).  Same for the output.
Matmuls run dj-major across the 8 PSUM banks of a group so each stationary is
loaded once per group.  Loads issue on the Sync HWDGE ring, stores on the
GpSimd SWDGE ring, PSUM eviction (+ fused scalar bias add) on the scalar
engine.  Inputs are cast to fp16 on host (PE runs 16-bit at full rate, PSUM
accumulates fp32; fp16 keeps 11 mantissa bits -> ~3e-4 rel err).
"""

import os
import sys

import numpy as np

for _p in ("/root/.axon_site/_ro/trn_rl_repo", "/opt/trn_rl_repo"):
    if os.path.isdir(_p) and _p not in sys.path:
        sys.path.append(_p)

H = W = 4096
KH = KW = 7
OH = OW = H - KH + 1            # 4090
NCORES = 8
CW = 512                        # output columns per core
CIN = CW + KW - 1               # 518 input columns per core (incl. halo)
BAND = 128 - (KH - 1)           # 122 output rows per band
NBANDS = -(-OH // BAND)         # 34
ROWS_PAD = BAND * (NBANDS - 1) + 128    # 4154 input rows incl. zero tail
GROUP = 8                       # bands per DMA batch / PSUM-bank rotation

_prog = None


def _program():
    global _prog
    if _prog is not None:
        return _prog

    from contextlib import ExitStack

    import concourse.bass as bass
    import concourse.tile as tile
    from concourse import bacc, mybir

    nc = bacc.Bacc("TRN2", target_bir_lowering=False, debug=False)
    xs = nc.dram_tensor(
        "xs", [128, NBANDS, CIN], mybir.dt.float16, kind="ExternalInput"
    )
    ab = nc.dram_tensor("ab", [128, KW, 128], mybir.dt.float16, kind="ExternalInput")
    br = nc.dram_tensor("br", [128, 1], mybir.dt.float32, kind="ExternalInput")
    # 128 rows per band (6 zero pad rows) so the store SBUF AP keeps a
    # power-of-2 partition count -- the DGE engine spray needs it
    yd = nc.dram_tensor(
        "yd", [128, NBANDS, CW], mybir.dt.float32, kind="ExternalOutput"
    )
    xs_ap, ab_ap, br_ap, yd_ap = xs.ap(), ab.ap(), br.ap(), yd.ap()

    with tile.TileContext(nc) as tc, ExitStack() as ctx:
        consts = ctx.enter_context(tc.tile_pool(name="consts", bufs=1))
        inp = ctx.enter_context(tc.tile_pool(name="inp", bufs=3))
        pss = ctx.enter_context(tc.tile_pool(name="pss", bufs=7, space="PSUM"))
        warm = ctx.enter_context(tc.tile_pool(name="warm", bufs=1, space="PSUM"))
        outp = ctx.enter_context(tc.tile_pool(name="outp", bufs=3))

        a_t = consts.tile([128, KW, 128], mybir.dt.float16)
        nc.sync.dma_start(a_t[:, :, :], ab_ap[:, :, :])
        b_t = consts.tile([128, 1], mybir.dt.float32)
        nc.sync.dma_start(b_t[:, :], br_ap)

        # Pre-warm the PE HAM clock gate during the load phase: ~10 dummy
        # matmuls (no data deps) keep the PE busy >3.4us so the real stream
        # starts at 2.4 GHz instead of 1.2 GHz.
        junk = consts.tile([128, 128 + CW], mybir.dt.float16)
        nc.gpsimd.memset(junk[:, :], 0)
        wps = warm.tile([128, CW], mybir.dt.float32)
        for _ in range(12):
            nc.tensor.matmul(
                wps[:, :],
                junk[:, 0:128],
                junk[:, 128 : 128 + CW],
                start=True,
                stop=True,
            )

        # small first groups so the PE starts earlier; small last group so the
        # final store + serial evictions don't hang off the kernel tail
        group_sizes = [1, 2, 4, 8, 8, 8, 3]
        assert sum(group_sizes) == NBANDS
        n_groups = len(group_sizes)
        b0 = 0
        for gi, g in enumerate(group_sizes):
            xin = inp.tile([128, GROUP, CIN], mybir.dt.float16, tag="xin")
            nc.sync.dma_start(xin[:, :g, :], xs_ap[:, b0 : b0 + g, :])

            yo = outp.tile([128, GROUP, CW], mybir.dt.float32, tag="yo")
            # dj-major over subgroups of <=4 bands: one LDWEIGHTS per dj per
            # subgroup, and at most 4 PSUM banks in flight (of 8) so slot
            # turnaround never gates the PE
            for s0 in range(0, g, 4):
                sg = min(4, g - s0)
                pst = [
                    pss.tile([128, CW], mybir.dt.float32, tag="ps", name=f"ps{b0}_{i}")
                    for i in range(s0, s0 + sg)
                ]
                for dj in range(KW):
                    for k, i in enumerate(range(s0, s0 + sg)):
                        nc.tensor.matmul(
                            pst[k][:, :],
                            a_t[:, dj, :],
                            xin[:, i, dj : dj + CW],
                            start=(dj == 0),
                            stop=(dj == KW - 1),
                        )
                for k, i in enumerate(range(s0, s0 + sg)):
                    # rows 122-127 are exact zeros (A columns >= BAND are zero)
                    nc.scalar.activation(
                        yo[:, i, :],
                        pst[k][:, :],
                        mybir.ActivationFunctionType.Identity,
                        bias=b_t[:, :],
                        scale=1.0,
                    )
            st_eng = nc.sync if gi == n_groups - 1 else nc.scalar
            st_eng.dma_start(yd_ap[:, b0 : b0 + g, :], yo[:, :g, :])
            b0 += g

    nc.compile()
    _prog = nc
    return nc


def _shards(x, weight, bias):
    x = np.asarray(x, dtype=np.float32)
    weight = np.asarray(weight, dtype=np.float32)
    bias = np.asarray(bias, dtype=np.float32)

    xp = np.zeros((ROWS_PAD, NCORES * CW + (KW - 1)), dtype=np.float16)
    xp[:H, :W] = x.astype(np.float16)

    wh = weight.astype(np.float16)
    abm = np.zeros((128, KW, 128), dtype=np.float16)
    idx = np.arange(BAND)
    for dj in range(KW):
        for di in range(KH):
            abm[idx + di, dj, idx] = wh[di, dj]

    brep = np.full((128, 1), np.float32(bias[0]), dtype=np.float32)

    s0, s1 = xp.strides
    ins = []
    for m in range(NCORES):
        core = xp[:, m * CW : m * CW + CIN]
        # xs[p, b, c] = core[BAND*b + p, c] -- overlapping-band strided view
        xb = np.lib.stride_tricks.as_strided(
            core, shape=(128, NBANDS, CIN), strides=(s0, BAND * s0, s1)
        )
        ins.append({"xs": np.ascontiguousarray(xb), "ab": abm, "br": brep})
    return ins


def _gather(results):
    y = np.empty((OH, OW), dtype=np.float32)
    for m in range(NCORES):
        c0 = m * CW
        c1 = min(c0 + CW, OW)
        # yd[r, b, c] = out[BAND*b + r, c] for r < BAND; rows >= BAND are pad
        full = results[m]["yd"].transpose(1, 0, 2)[:, :BAND, :].reshape(
            BAND * NBANDS, CW
        )
        y[:, c0:c1] = full[:OH, : c1 - c0]
    return y


def kernel(x, weight, bias):
    from concourse.bass_utils import run_bass_kernel_spmd

    nc = _program()
    in_maps = _shards(x, weight, bias)
    res = run_bass_kernel_spmd(nc, in_maps, core_ids=list(range(NCORES)))
    return _gather(res.results)



# revision 4
# speedup vs baseline: 1.4534x; 1.4534x over previous
"""Trainium2 Bass kernel: 4096x4096 fp32 image, 7x7 valid cross-correlation + bias.

Strategy
--------
Column-shard the image across 8 NeuronCores: core m computes output columns
[512*m, 512*m+512) (tail cropped on host; every core sees 512 columns + 6 halo
columns of input, zero-padded to 520).

2D-tiled Toeplitz: the 128 SBUF partitions carry a 16x8 image patch,
p = 8*a + b  <->  x[16*s + a, 8*q + b]  (slab s along the free axis, q-tile of
8 columns).  One matmul contracts a full patch against a stationary
S[(a,b), (i,j)] = w[a-i, b-j] producing 128 output pixels (i,j) per streamed
column -- 2x the useful density of the 1D banded-Toeplitz form.  Taps that
cross the patch boundary (i+di >= 16 row-wrap, j+dj >= 8 col-wrap) are handled
by three more matmuls whose moving operand is the same SBUF buffer shifted by
one slab (free offset +65) and/or one q-tile (free offset +1), accumulated in
the same PSUM bank via start/stop.  Total: 4 matmuls of 512 free-columns per
128x512 output chunk = 2048 PE cycles per 65536 outputs, ~2x faster than the
7-matmul row-band form.

Outputs are evicted PSUM->SBUF with a fused bias add (scalar/vector engines
alternate) and stored as fp16 (halves store traffic; |err| ~ 5e-4 rel, gate is
2e-2).  Inputs are fp16 (PE streams 16-bit at full rate, PSUM accumulates
fp32).  Loads ride the Sync HWDGE ring, stores the GpSimd ring.
"""

import os
import sys

import numpy as np

for _p in ("/root/.axon_site/_ro/trn_rl_repo", "/opt/trn_rl_repo"):
    if os.path.isdir(_p) and _p not in sys.path:
        sys.path.append(_p)

H = W = 4096
KH = KW = 7
OH = OW = H - KH + 1            # 4090
NCORES = 8
CW = 512                        # output columns per core
A, B = 16, 8                    # patch rows x cols (A*B = 128 partitions)
QT = 65                         # q-tiles per slab (65*8 = 520 >= 512+6)
NSLAB = 257                     # 16-row slabs (4112 rows incl. zero tail)
ROWS_PAD = NSLAB * A            # 4112
COLS_PAD = QT * B               # 520
SPC = 8                         # slabs per chunk (128 output rows)
NCHUNK = 32                     # chunks per core
NGRP = 4                        # chunk groups (8 PSUM banks each)
NWARM = 10                      # HAM warm-up matmuls

_prog = None


def _program():
    global _prog
    if _prog is not None:
        return _prog

    from contextlib import ExitStack

    import concourse.bass as bass
    import concourse.tile as tile
    from concourse import bacc, mybir

    nc = bacc.Bacc("TRN2", target_bir_lowering=False, debug=False)
    xs = nc.dram_tensor(
        "xs", [128, NSLAB, QT], mybir.dt.float16, kind="ExternalInput"
    )
    ws = nc.dram_tensor("ws", [128, 4, 128], mybir.dt.float16, kind="ExternalInput")
    br = nc.dram_tensor("br", [128, 1], mybir.dt.float32, kind="ExternalInput")
    yd = nc.dram_tensor(
        "yd", [128, NCHUNK, CW], mybir.dt.float16, kind="ExternalOutput"
    )
    xs_ap, ws_ap, br_ap, yd_ap = xs.ap(), ws.ap(), br.ap(), yd.ap()

    with tile.TileContext(nc) as tc, ExitStack() as ctx:
        consts = ctx.enter_context(tc.tile_pool(name="consts", bufs=1))
        xpool = ctx.enter_context(tc.tile_pool(name="xpool", bufs=1))
        pss = ctx.enter_context(tc.tile_pool(name="pss", bufs=8, space="PSUM"))
        ypool = ctx.enter_context(tc.tile_pool(name="ypool", bufs=1))

        w_t = consts.tile([128, 4, 128], mybir.dt.float16)
        nc.sync.dma_start(w_t[:, :, :], ws_ap[:, :, :])
        b_t = consts.tile([128, 1], mybir.dt.float32)
        nc.sync.dma_start(b_t[:, :], br_ap)

        # Pre-warm the PE HAM clock gate while the first loads stream: dummy
        # matmuls (no data deps) push the PE past the ~3.4us activity window
        # so the real stream runs at 2.4 GHz instead of 1.2 GHz.
        junk = consts.tile([128, 128 + CW], mybir.dt.float16)
        nc.gpsimd.memset(junk[:, :], 0)
        wps = pss.tile([128, SPC, 64], mybir.dt.float32, tag="ps", name="warm")
        for _ in range(NWARM):
            nc.tensor.matmul(
                wps[:, :, :],
                junk[:, 0:128],
                junk[:, 128 : 128 + CW],
                start=True,
                stop=True,
            )

        # whole per-core input lives in SBUF (33.4 KB/partition); load in
        # 16-slab granules so early chunks start before the stream finishes
        xall = xpool.tile([128, NSLAB, QT], mybir.dt.float16)
        for s0 in range(0, NSLAB, 16):
            s1 = min(s0 + 16, NSLAB)
            nc.sync.dma_start(xall[:, s0:s1, :], xs_ap[:, s0:s1, :])

        yo = ypool.tile([128, NCHUNK, CW], mybir.dt.float16)

        for g in range(NGRP):
            pts = [
                pss.tile([128, SPC, 64], mybir.dt.float32, tag="ps", name=f"ps{g}_{k}")
                for k in range(8)
            ]
            # stationary-major: one weight set streams 8 chunks before the
            # next LDWEIGHTS; the 4 passes (row-wrap x col-wrap) accumulate
            # into each chunk's PSUM bank
            for si, (dt, dq) in enumerate([(0, 0), (0, 1), (1, 0), (1, 1)]):
                for k in range(8):
                    c = g * 8 + k
                    s0 = c * SPC + dt
                    nc.tensor.matmul(
                        pts[k][:, :, :],
                        w_t[:, si, :],
                        xall[:, s0 : s0 + SPC, dq : dq + 64],
                        start=(si == 0),
                        stop=(si == 3),
                    )
            for k in range(8):
                c = g * 8 + k
                if k % 2 == 0:
                    nc.scalar.activation(
                        yo[:, c, :],
                        pts[k][:, :, :],
                        mybir.ActivationFunctionType.Identity,
                        bias=b_t[:, :],
                        scale=1.0,
                    )
                else:
                    nc.vector.tensor_scalar_add(
                        yo[:, c, :], pts[k][:, :, :], b_t[:, :]
                    )
                nc.gpsimd.dma_start(yd_ap[:, c, :], yo[:, c, :])

    nc.compile()
    _prog = nc
    return nc


def _shards(x, weight, bias):
    x = np.asarray(x, dtype=np.float32)
    weight = np.asarray(weight, dtype=np.float32)
    bias = np.asarray(bias, dtype=np.float32)

    xh = x.astype(np.float16)
    wh = weight.astype(np.float16)

    # stationaries: S[si=(2dt+dq)][8a+b, 8i+j] = w[a+16dt-i, b+8dq-j]
    S = np.zeros((128, 4, 128), dtype=np.float16)
    aa, bb, ii, jj = np.meshgrid(
        np.arange(A), np.arange(B), np.arange(A), np.arange(B), indexing="ij"
    )
    for si, (dt, dq) in enumerate([(0, 0), (0, 1), (1, 0), (1, 1)]):
        di = aa + 16 * dt - ii
        dj = bb + 8 * dq - jj
        m = (di >= 0) & (di < KH) & (dj >= 0) & (dj < KW)
        S[(aa * B + bb)[m], si, (ii * B + jj)[m]] = wh[di[m], dj[m]]

    brep = np.full((128, 1), np.float32(bias[0]), dtype=np.float32)

    ins = []
    for m in range(NCORES):
        xpad = np.zeros((ROWS_PAD, COLS_PAD), dtype=np.float16)
        c0 = m * CW
        c1 = min(c0 + CW + KW - 1, W)
        xpad[:H, : c1 - c0] = xh[:, c0:c1]
        # xs[8a+b, s, q] = xpad[16s+a, 8q+b]
        xsm = np.ascontiguousarray(
            xpad.reshape(NSLAB, A, QT, B).transpose(1, 3, 0, 2).reshape(128, NSLAB, QT)
        )
        ins.append({"xs": xsm, "ws": S, "br": brep})
    return ins


def _gather(results):
    y = np.empty((OH, OW), dtype=np.float32)
    for m in range(NCORES):
        c0 = m * CW
        c1 = min(c0 + CW, OW)
        # yd[8i+j, c, 64s+q] = out[16(8c+s)+i, 8q+j]
        full = (
            results[m]["yd"]
            .reshape(A, B, NCHUNK, SPC, 64)
            .transpose(2, 3, 0, 4, 1)
            .reshape(ROWS_PAD - A, CW)
        )
        y[:, c0:c1] = full[:OH, : c1 - c0].astype(np.float32)
    return y


def kernel(x, weight, bias):
    from concourse.bass_utils import run_bass_kernel_spmd

    nc = _program()
    in_maps = _shards(x, weight, bias)
    res = run_bass_kernel_spmd(nc, in_maps, core_ids=list(range(NCORES)))
    return _gather(res.results)


# revision 7
# speedup vs baseline: 1.4962x; 1.0294x over previous
"""Trainium2 Bass kernel: 4096x4096 fp32 image, 7x7 valid cross-correlation + bias.

Strategy
--------
Column-shard the image across 8 NeuronCores: core m computes output columns
[512*m, 512*m+512) (tail cropped on host; every core sees 512 columns + 6 halo
columns of input, zero-padded to 520).

2D-tiled Toeplitz: the 128 SBUF partitions carry a 16x8 image patch,
p = 8*a + b  <->  x[16*s + a, 8*q + b]  (slab s along the free axis, q-tile of
8 columns).  One matmul contracts a full patch against a stationary
S[(a,b), (i,j)] = w[a-i, b-j] producing 128 output pixels (i,j) per streamed
column -- 2x the useful density of the 1D banded-Toeplitz form.  Taps that
cross the patch boundary (i+di >= 16 row-wrap, j+dj >= 8 col-wrap) are handled
by three more matmuls whose moving operand is the same SBUF buffer shifted by
one slab (free offset +65) and/or one q-tile (free offset +1), accumulated in
the same PSUM bank via start/stop.  Total: 4 matmuls of 512 free-columns per
128x512 output chunk = 2048 PE cycles per 65536 outputs, ~2x faster than the
7-matmul row-band form.

Outputs are evicted PSUM->SBUF with a fused bias add (scalar/vector engines
alternate) and stored as fp16 (halves store traffic; |err| ~ 5e-4 rel, gate is
2e-2).  Inputs are fp16 (PE streams 16-bit at full rate, PSUM accumulates
fp32).  Loads ride the Sync HWDGE ring, stores the GpSimd ring.
"""

import os
import sys

import numpy as np

for _p in ("/root/.axon_site/_ro/trn_rl_repo", "/opt/trn_rl_repo"):
    if os.path.isdir(_p) and _p not in sys.path:
        sys.path.append(_p)

H = W = 4096
KH = KW = 7
OH = OW = H - KH + 1            # 4090
NCORES = 8
CW = 512                        # output columns per core
A, B = 16, 8                    # patch rows x cols (A*B = 128 partitions)
QT = 65                         # q-tiles per slab (65*8 = 520 >= 512+6)
NSLAB = 257                     # 16-row slabs (4112 rows incl. zero tail)
ROWS_PAD = NSLAB * A            # 4112
COLS_PAD = QT * B               # 520
SPC = 8                         # slabs per chunk (128 output rows)
NCHUNK = 32                     # chunks per core
NGRP = 4                        # chunk groups (8 PSUM banks each)
LOADG = 8                       # slabs per input DMA granule
STOREB = [4, 4, 4, 4, 4, 4, 4, 3, 1]   # chunks per output DMA (tiny tail)

_prog = None


def _program():
    global _prog
    if _prog is not None:
        return _prog

    from contextlib import ExitStack

    import concourse.bass as bass
    import concourse.tile as tile
    from concourse import bacc, mybir

    nc = bacc.Bacc("TRN2", target_bir_lowering=False, debug=False)
    xs = nc.dram_tensor(
        "xs", [128, NSLAB, QT], mybir.dt.float16, kind="ExternalInput"
    )
    ws = nc.dram_tensor("ws", [128, 4, 128], mybir.dt.float16, kind="ExternalInput")
    br = nc.dram_tensor("br", [128, 1], mybir.dt.float32, kind="ExternalInput")
    yd = nc.dram_tensor(
        "yd", [128, NCHUNK, CW], mybir.dt.float16, kind="ExternalOutput"
    )
    xs_ap, ws_ap, br_ap, yd_ap = xs.ap(), ws.ap(), br.ap(), yd.ap()

    with tile.TileContext(nc) as tc, ExitStack() as ctx:
        consts = ctx.enter_context(tc.tile_pool(name="consts", bufs=1))
        xpool = ctx.enter_context(tc.tile_pool(name="xpool", bufs=1))
        pss = ctx.enter_context(tc.tile_pool(name="pss", bufs=8, space="PSUM"))
        ypool = ctx.enter_context(tc.tile_pool(name="ypool", bufs=1))

        w_t = consts.tile([128, 4, 128], mybir.dt.float16)
        nc.sync.dma_start(w_t[:, :, :], ws_ap[:, :, :])
        b_t = consts.tile([128, 1], mybir.dt.float32)
        nc.sync.dma_start(b_t[:, :], br_ap)

        # whole per-core input lives in SBUF (33.4 KB/partition); small load
        # granules so chunk 0's matmuls start as soon as possible -- group
        # 0's first pass doubles as the HAM warm-up (runs cold at 1.2 GHz,
        # paced by the load stream either way)
        xall = xpool.tile([128, NSLAB, QT], mybir.dt.float16)
        for s0 in range(0, NSLAB, LOADG):
            s1 = min(s0 + LOADG, NSLAB)
            nc.sync.dma_start(xall[:, s0:s1, :], xs_ap[:, s0:s1, :])

        yo = ypool.tile([128, NCHUNK, CW], mybir.dt.float16)
        stores = []
        c_acc = 0
        for nb in STOREB:
            stores.append((c_acc, c_acc + nb))
            c_acc += nb

        for g in range(NGRP):
            pts = [
                pss.tile([128, SPC, 64], mybir.dt.float32, tag="ps", name=f"ps{g}_{k}")
                for k in range(8)
            ]
            # stationary-major: one weight set streams 8 chunks before the
            # next LDWEIGHTS; the 4 passes (row-wrap x col-wrap) accumulate
            # into each chunk's PSUM bank
            for si, (dt, dq) in enumerate([(0, 0), (0, 1), (1, 0), (1, 1)]):
                for k in range(8):
                    c = g * 8 + k
                    s0 = c * SPC + dt
                    nc.tensor.matmul(
                        pts[k][:, :, :],
                        w_t[:, si, :],
                        xall[:, s0 : s0 + SPC, dq : dq + 64],
                        start=(si == 0),
                        stop=(si == 3),
                    )
            for k in range(8):
                c = g * 8 + k
                if k % 2 == 0:
                    nc.scalar.activation(
                        yo[:, c, :],
                        pts[k][:, :, :],
                        mybir.ActivationFunctionType.Identity,
                        bias=b_t[:, :],
                        scale=1.0,
                    )
                else:
                    nc.vector.tensor_scalar_add(
                        yo[:, c, :], pts[k][:, :, :], b_t[:, :]
                    )
                while stores and stores[0][1] == c + 1:
                    c0, c1 = stores.pop(0)
                    st_eng = nc.sync if not stores else nc.gpsimd
                    st_eng.dma_start(yd_ap[:, c0:c1, :], yo[:, c0:c1, :])

    nc.compile()
    _prog = nc
    return nc


def _shards(x, weight, bias):
    x = np.asarray(x, dtype=np.float32)
    weight = np.asarray(weight, dtype=np.float32)
    bias = np.asarray(bias, dtype=np.float32)

    xh = x.astype(np.float16)
    wh = weight.astype(np.float16)

    # stationaries: S[si=(2dt+dq)][8a+b, 8i+j] = w[a+16dt-i, b+8dq-j]
    S = np.zeros((128, 4, 128), dtype=np.float16)
    aa, bb, ii, jj = np.meshgrid(
        np.arange(A), np.arange(B), np.arange(A), np.arange(B), indexing="ij"
    )
    for si, (dt, dq) in enumerate([(0, 0), (0, 1), (1, 0), (1, 1)]):
        di = aa + 16 * dt - ii
        dj = bb + 8 * dq - jj
        m = (di >= 0) & (di < KH) & (dj >= 0) & (dj < KW)
        S[(aa * B + bb)[m], si, (ii * B + jj)[m]] = wh[di[m], dj[m]]

    brep = np.full((128, 1), np.float32(bias[0]), dtype=np.float32)

    ins = []
    for m in range(NCORES):
        xpad = np.zeros((ROWS_PAD, COLS_PAD), dtype=np.float16)
        c0 = m * CW
        c1 = min(c0 + CW + KW - 1, W)
        xpad[:H, : c1 - c0] = xh[:, c0:c1]
        # xs[8a+b, s, q] = xpad[16s+a, 8q+b]
        xsm = np.ascontiguousarray(
            xpad.reshape(NSLAB, A, QT, B).transpose(1, 3, 0, 2).reshape(128, NSLAB, QT)
        )
        ins.append({"xs": xsm, "ws": S, "br": brep})
    return ins


def _gather(results):
    y = np.empty((OH, OW), dtype=np.float32)
    for m in range(NCORES):
        c0 = m * CW
        c1 = min(c0 + CW, OW)
        # yd[8i+j, c, 64s+q] = out[16(8c+s)+i, 8q+j]
        full = (
            results[m]["yd"]
            .reshape(A, B, NCHUNK, SPC, 64)
            .transpose(2, 3, 0, 4, 1)
            .reshape(ROWS_PAD - A, CW)
        )
        y[:, c0:c1] = full[:OH, : c1 - c0].astype(np.float32)
    return y


def kernel(x, weight, bias):
    from concourse.bass_utils import run_bass_kernel_spmd

    nc = _program()
    in_maps = _shards(x, weight, bias)
    res = run_bass_kernel_spmd(nc, in_maps, core_ids=list(range(NCORES)))
    return _gather(res.results)


# revision 13
# speedup vs baseline: 1.5378x; 1.0278x over previous
"""Trainium2 Bass kernel: 4096x4096 fp32 image, 7x7 valid cross-correlation + bias.

Strategy
--------
Column-shard the image across 8 NeuronCores: core m computes output columns
[512*m, 512*m+512) (tail cropped on host; every core sees 512 columns + 6 halo
columns of input, zero-padded to 520).

2D-tiled Toeplitz: the 128 SBUF partitions carry a 16x8 image patch,
p = 8*a + b  <->  x[16*s + a, 8*q + b]  (slab s along the free axis, q-tile of
8 columns).  One matmul contracts a full patch against a stationary
S[(a,b), (i,j)] = w[a-i, b-j] producing 128 output pixels (i,j) per streamed
column -- 2x the useful density of the 1D banded-Toeplitz form.  Taps that
cross the patch boundary (i+di >= 16 row-wrap, j+dj >= 8 col-wrap) are handled
by three more matmuls whose moving operand is the same SBUF buffer shifted by
one slab (free offset +65) and/or one q-tile (free offset +1), accumulated in
the same PSUM bank via start/stop.  Total: 4 matmuls of 512 free-columns per
128x512 output chunk = 2048 PE cycles per 65536 outputs, ~2x faster than the
7-matmul row-band form.

Outputs are evicted PSUM->SBUF with a fused bias add (scalar/vector engines
alternate) and stored as fp16 (halves store traffic; |err| ~ 5e-4 rel, gate is
2e-2).  Inputs are fp16 (PE streams 16-bit at full rate, PSUM accumulates
fp32).  Loads ride the Sync HWDGE ring, stores the GpSimd ring.
"""

import os
import sys

import numpy as np

for _p in ("/root/.axon_site/_ro/trn_rl_repo", "/opt/trn_rl_repo"):
    if os.path.isdir(_p) and _p not in sys.path:
        sys.path.append(_p)

H = W = 4096
KH = KW = 7
OH = OW = H - KH + 1            # 4090
NCORES = 8
CW = 512                        # output columns per core
A, B = 16, 8                    # patch rows x cols (A*B = 128 partitions)
QT = 65                         # q-tiles per slab (65*8 = 520 >= 512+6)
NSLAB = 257                     # 16-row slabs (4112 rows incl. zero tail)
ROWS_PAD = NSLAB * A            # 4112
COLS_PAD = QT * B               # 520
SPC = 8                         # slabs per chunk (128 output rows)
NCHUNK = 32                     # chunks per core
NGRP = 4                        # chunk groups (8 PSUM banks each)
# chunks per output DMA: batches of 4 while compute hides them, per-chunk at
# the tail so the last store is tiny
STOREB = [4, 4, 4, 4, 4, 4, 4, 1, 1, 1, 1]

_prog = None


def _program():
    global _prog
    if _prog is not None:
        return _prog

    from contextlib import ExitStack

    import concourse.bass as bass
    import concourse.tile as tile
    from concourse import bacc, mybir

    nc = bacc.Bacc("TRN2", target_bir_lowering=False, debug=False)
    xs = nc.dram_tensor(
        "xs", [128, NSLAB, QT], mybir.dt.float16, kind="ExternalInput"
    )
    ws = nc.dram_tensor("ws", [128, 4, 128], mybir.dt.float16, kind="ExternalInput")
    br = nc.dram_tensor("br", [128, 1], mybir.dt.float32, kind="ExternalInput")
    yd = nc.dram_tensor(
        "yd", [128, NCHUNK, CW], mybir.dt.float16, kind="ExternalOutput"
    )
    xs_ap, ws_ap, br_ap, yd_ap = xs.ap(), ws.ap(), br.ap(), yd.ap()

    with tile.TileContext(nc) as tc, ExitStack() as ctx:
        consts = ctx.enter_context(tc.tile_pool(name="consts", bufs=1))
        xpool = ctx.enter_context(tc.tile_pool(name="xpool", bufs=1))
        pss = ctx.enter_context(tc.tile_pool(name="pss", bufs=8, space="PSUM"))
        ypool = ctx.enter_context(tc.tile_pool(name="ypool", bufs=1))

        # every dma_start costs ~620ns of issuing-engine time (descriptor
        # generation) plus ~1.5us trigger-to-data latency, so the startup
        # triggers are spread across the three DMA-capable engine queues
        # (sync/scalar/gpsimd) and the rest of the image streams on sync in
        # 16-slab granules.
        w_t = consts.tile([128, 4, 128], mybir.dt.float16)
        nc.scalar.dma_start(w_t[:, :, :], ws_ap[:, :, :])
        b_t = consts.tile([128, 1], mybir.dt.float32)

        junk = consts.tile([128, 128 + CW], mybir.dt.float16)
        nc.gpsimd.memset(junk[:, :], 0)

        xall = xpool.tile([128, NSLAB, QT], mybir.dt.float16)
        nc.sync.dma_start(xall[:, 0:8, :], xs_ap[:, 0:8, :])
        nc.gpsimd.dma_start(xall[:, 8:16, :], xs_ap[:, 8:16, :])
        nc.scalar.dma_start(xall[:, 16:24, :], xs_ap[:, 16:24, :])
        nc.scalar.dma_start(b_t[:, :], br_ap)
        for s0 in range(24, NSLAB, 16):
            s1 = min(s0 + 16, NSLAB)
            nc.sync.dma_start(xall[:, s0:s1, :], xs_ap[:, s0:s1, :])

        # burn the HAM cold window (~3.4us at 1.2 GHz) with a few junk
        # matmuls while the first granule's DMA is still in flight
        wps = pss.tile([128, SPC, 64], mybir.dt.float32, tag="ps", name="warm")
        for _ in range(4):
            nc.tensor.matmul(
                wps[:, :, :],
                junk[:, 0:128],
                junk[:, 128 : 128 + CW],
                start=True,
                stop=True,
            )

        yo = ypool.tile([128, NCHUNK, CW], mybir.dt.float16)
        stores = []
        c_acc = 0
        for nb in STOREB:
            stores.append((c_acc, c_acc + nb))
            c_acc += nb

        # chunk-major: the 4 passes (row-wrap x col-wrap) of a chunk
        # accumulate back-to-back into its PSUM bank (canonical K-tiled
        # accumulate); LDWEIGHTS for the next pass hides in the background
        # weight buffer.  Evictions alternate scalar/vector.
        for c in range(NCHUNK):
            pt = pss.tile([128, SPC, 64], mybir.dt.float32, tag="ps", name=f"ps{c}")
            for si, (dt, dq) in enumerate([(0, 0), (0, 1), (1, 0), (1, 1)]):
                s0 = c * SPC + dt
                nc.tensor.matmul(
                    pt[:, :, :],
                    w_t[:, si, :],
                    xall[:, s0 : s0 + SPC, dq : dq + 64],
                    start=(si == 0),
                    stop=(si == 3),
                )
            if c % 2 == 0:
                nc.scalar.activation(
                    yo[:, c, :],
                    pt[:, :, :],
                    mybir.ActivationFunctionType.Identity,
                    bias=b_t[:, :],
                    scale=1.0,
                )
            else:
                nc.vector.tensor_scalar_add(yo[:, c, :], pt[:, :, :], b_t[:, :])
            while stores and stores[0][1] == c + 1:
                c0, c1 = stores.pop(0)
                st_eng = nc.sync if not stores else nc.gpsimd
                st_eng.dma_start(yd_ap[:, c0:c1, :], yo[:, c0:c1, :])


    nc.compile()
    _prog = nc
    return nc


def _shards(x, weight, bias):
    x = np.asarray(x, dtype=np.float32)
    weight = np.asarray(weight, dtype=np.float32)
    bias = np.asarray(bias, dtype=np.float32)

    xh = x.astype(np.float16)
    wh = weight.astype(np.float16)

    # stationaries: S[si=(2dt+dq)][8a+b, 8i+j] = w[a+16dt-i, b+8dq-j]
    S = np.zeros((128, 4, 128), dtype=np.float16)
    aa, bb, ii, jj = np.meshgrid(
        np.arange(A), np.arange(B), np.arange(A), np.arange(B), indexing="ij"
    )
    for si, (dt, dq) in enumerate([(0, 0), (0, 1), (1, 0), (1, 1)]):
        di = aa + 16 * dt - ii
        dj = bb + 8 * dq - jj
        m = (di >= 0) & (di < KH) & (dj >= 0) & (dj < KW)
        S[(aa * B + bb)[m], si, (ii * B + jj)[m]] = wh[di[m], dj[m]]

    brep = np.full((128, 1), np.float32(bias[0]), dtype=np.float32)

    ins = []
    for m in range(NCORES):
        xpad = np.zeros((ROWS_PAD, COLS_PAD), dtype=np.float16)
        c0 = m * CW
        c1 = min(c0 + CW + KW - 1, W)
        xpad[:H, : c1 - c0] = xh[:, c0:c1]
        # xs[8a+b, s, q] = xpad[16s+a, 8q+b]
        xsm = np.ascontiguousarray(
            xpad.reshape(NSLAB, A, QT, B).transpose(1, 3, 0, 2).reshape(128, NSLAB, QT)
        )
        ins.append({"xs": xsm, "ws": S, "br": brep})
    return ins


def _gather(results):
    y = np.empty((OH, OW), dtype=np.float32)
    for m in range(NCORES):
        c0 = m * CW
        c1 = min(c0 + CW, OW)
        # yd[8i+j, c, 64s+q] = out[16(8c+s)+i, 8q+j]
        full = (
            results[m]["yd"]
            .reshape(A, B, NCHUNK, SPC, 64)
            .transpose(2, 3, 0, 4, 1)
            .reshape(ROWS_PAD - A, CW)
        )
        y[:, c0:c1] = full[:OH, : c1 - c0].astype(np.float32)
    return y


def kernel(x, weight, bias):
    from concourse.bass_utils import run_bass_kernel_spmd

    nc = _program()
    in_maps = _shards(x, weight, bias)
    res = run_bass_kernel_spmd(nc, in_maps, core_ids=list(range(NCORES)))
    return _gather(res.results)


# revision 14
# speedup vs baseline: 1.5942x; 1.0367x over previous
"""Trainium2 Bass kernel: 4096x4096 fp32 image, 7x7 valid cross-correlation + bias.

Strategy
--------
Column-shard the image across 8 NeuronCores: core m computes output columns
[512*m, 512*m+512) (tail cropped on host; every core sees 512 columns + 6 halo
columns of input, zero-padded to 520).

2D-tiled Toeplitz: the 128 SBUF partitions carry a 16x8 image patch,
p = 8*a + b  <->  x[16*s + a, 8*q + b]  (slab s along the free axis, q-tile of
8 columns).  One matmul contracts a full patch against a stationary
S[(a,b), (i,j)] = w[a-i, b-j] producing 128 output pixels (i,j) per streamed
column -- 2x the useful density of the 1D banded-Toeplitz form.  Taps that
cross the patch boundary (i+di >= 16 row-wrap, j+dj >= 8 col-wrap) are handled
by three more matmuls whose moving operand is the same SBUF buffer shifted by
one slab (free offset +65) and/or one q-tile (free offset +1), accumulated in
the same PSUM bank via start/stop.  Total: 4 matmuls of 512 free-columns per
128x512 output chunk = 2048 PE cycles per 65536 outputs, ~2x faster than the
7-matmul row-band form.

Outputs are evicted PSUM->SBUF with a fused bias add (scalar/vector engines
alternate) and stored as fp16 (halves store traffic; |err| ~ 5e-4 rel, gate is
2e-2).  Inputs are fp16 (PE streams 16-bit at full rate, PSUM accumulates
fp32).  Loads ride the Sync HWDGE ring, stores the GpSimd ring.
"""

import os
import sys

import numpy as np

for _p in ("/root/.axon_site/_ro/trn_rl_repo", "/opt/trn_rl_repo"):
    if os.path.isdir(_p) and _p not in sys.path:
        sys.path.append(_p)

H = W = 4096
KH = KW = 7
OH = OW = H - KH + 1            # 4090
NCORES = 8
CW = 512                        # output columns per core
A, B = 16, 8                    # patch rows x cols (A*B = 128 partitions)
QT = 65                         # q-tiles per slab (65*8 = 520 >= 512+6)
NSLAB = 257                     # 16-row slabs (4112 rows incl. zero tail)
ROWS_PAD = NSLAB * A            # 4112
COLS_PAD = QT * B               # 520
SPC = 8                         # slabs per chunk (128 output rows)
NCHUNK = 32                     # chunks per core
NGRP = 4                        # chunk groups (8 PSUM banks each)
# chunks per output DMA: batches of 4 while compute hides them, per-chunk at
# the tail so the last store is tiny
STOREB = [4, 4, 4, 4, 4, 4, 4, 1, 1, 1, 1]

_prog = None


def _program():
    global _prog
    if _prog is not None:
        return _prog

    from contextlib import ExitStack

    import concourse.bass as bass
    import concourse.tile as tile
    from concourse import bacc, mybir

    nc = bacc.Bacc("TRN2", target_bir_lowering=False, debug=False)
    xs = nc.dram_tensor(
        "xs", [128, NSLAB, QT], mybir.dt.float16, kind="ExternalInput"
    )
    ws = nc.dram_tensor("ws", [128, 4, 128], mybir.dt.float16, kind="ExternalInput")
    br = nc.dram_tensor("br", [128, 1], mybir.dt.float32, kind="ExternalInput")
    yd = nc.dram_tensor(
        "yd", [128, NCHUNK, CW], mybir.dt.float16, kind="ExternalOutput"
    )
    xs_ap, ws_ap, br_ap, yd_ap = xs.ap(), ws.ap(), br.ap(), yd.ap()

    with tile.TileContext(nc) as tc, ExitStack() as ctx:
        consts = ctx.enter_context(tc.tile_pool(name="consts", bufs=1))
        xpool = ctx.enter_context(tc.tile_pool(name="xpool", bufs=1))
        pss = ctx.enter_context(tc.tile_pool(name="pss", bufs=8, space="PSUM"))
        ypool = ctx.enter_context(tc.tile_pool(name="ypool", bufs=1))

        # every dma_start costs ~620ns of issuing-engine time (descriptor
        # generation) plus ~1.5us trigger-to-data latency, so the startup
        # triggers are spread across the three DMA-capable engine queues
        # (sync/scalar/gpsimd) and the rest of the image streams on sync in
        # 16-slab granules.
        w_t = consts.tile([128, 4, 128], mybir.dt.float16)
        nc.scalar.dma_start(w_t[:, :, :], ws_ap[:, :, :])
        b_t = consts.tile([128, 1], mybir.dt.float32)

        nc.scalar.dma_start(b_t[:, :], br_ap)

        # all image loads on ONE ring (sync) in strict need order -- a
        # second ring's granule gets starved behind a deep prefetch stream
        # and stalls the PE cold.  Small granules first so chunk 0 starts
        # ~8.3us; the first ~8 real matmuls double as the HAM warm-up.
        xall = xpool.tile([128, NSLAB, QT], mybir.dt.float16)
        sched = [(0, 8), (8, 16), (16, 24)]
        sched += [(s0, min(s0 + 16, NSLAB)) for s0 in range(24, NSLAB, 16)]
        for s0, s1 in sched:
            nc.sync.dma_start(xall[:, s0:s1, :], xs_ap[:, s0:s1, :])

        yo = ypool.tile([128, NCHUNK, CW], mybir.dt.float16)
        stores = []
        c_acc = 0
        for nb in STOREB:
            stores.append((c_acc, c_acc + nb))
            c_acc += nb

        # chunk-major: the 4 passes (row-wrap x col-wrap) of a chunk
        # accumulate back-to-back into its PSUM bank (canonical K-tiled
        # accumulate); LDWEIGHTS for the next pass hides in the background
        # weight buffer.  Evictions alternate scalar/vector.
        for c in range(NCHUNK):
            pt = pss.tile([128, SPC, 64], mybir.dt.float32, tag="ps", name=f"ps{c}")
            for si, (dt, dq) in enumerate([(0, 0), (0, 1), (1, 0), (1, 1)]):
                s0 = c * SPC + dt
                nc.tensor.matmul(
                    pt[:, :, :],
                    w_t[:, si, :],
                    xall[:, s0 : s0 + SPC, dq : dq + 64],
                    start=(si == 0),
                    stop=(si == 3),
                )
            if c % 2 == 0:
                nc.scalar.activation(
                    yo[:, c, :],
                    pt[:, :, :],
                    mybir.ActivationFunctionType.Identity,
                    bias=b_t[:, :],
                    scale=1.0,
                )
            else:
                nc.vector.tensor_scalar_add(yo[:, c, :], pt[:, :, :], b_t[:, :])
            while stores and stores[0][1] == c + 1:
                c0, c1 = stores.pop(0)
                st_eng = nc.sync if not stores else nc.gpsimd
                st_eng.dma_start(yd_ap[:, c0:c1, :], yo[:, c0:c1, :])


    nc.compile()
    _prog = nc
    return nc


def _shards(x, weight, bias):
    x = np.asarray(x, dtype=np.float32)
    weight = np.asarray(weight, dtype=np.float32)
    bias = np.asarray(bias, dtype=np.float32)

    xh = x.astype(np.float16)
    wh = weight.astype(np.float16)

    # stationaries: S[si=(2dt+dq)][8a+b, 8i+j] = w[a+16dt-i, b+8dq-j]
    S = np.zeros((128, 4, 128), dtype=np.float16)
    aa, bb, ii, jj = np.meshgrid(
        np.arange(A), np.arange(B), np.arange(A), np.arange(B), indexing="ij"
    )
    for si, (dt, dq) in enumerate([(0, 0), (0, 1), (1, 0), (1, 1)]):
        di = aa + 16 * dt - ii
        dj = bb + 8 * dq - jj
        m = (di >= 0) & (di < KH) & (dj >= 0) & (dj < KW)
        S[(aa * B + bb)[m], si, (ii * B + jj)[m]] = wh[di[m], dj[m]]

    brep = np.full((128, 1), np.float32(bias[0]), dtype=np.float32)

    ins = []
    for m in range(NCORES):
        xpad = np.zeros((ROWS_PAD, COLS_PAD), dtype=np.float16)
        c0 = m * CW
        c1 = min(c0 + CW + KW - 1, W)
        xpad[:H, : c1 - c0] = xh[:, c0:c1]
        # xs[8a+b, s, q] = xpad[16s+a, 8q+b]
        xsm = np.ascontiguousarray(
            xpad.reshape(NSLAB, A, QT, B).transpose(1, 3, 0, 2).reshape(128, NSLAB, QT)
        )
        ins.append({"xs": xsm, "ws": S, "br": brep})
    return ins


def _gather(results):
    y = np.empty((OH, OW), dtype=np.float32)
    for m in range(NCORES):
        c0 = m * CW
        c1 = min(c0 + CW, OW)
        # yd[8i+j, c, 64s+q] = out[16(8c+s)+i, 8q+j]
        full = (
            results[m]["yd"]
            .reshape(A, B, NCHUNK, SPC, 64)
            .transpose(2, 3, 0, 4, 1)
            .reshape(ROWS_PAD - A, CW)
        )
        y[:, c0:c1] = full[:OH, : c1 - c0].astype(np.float32)
    return y


def kernel(x, weight, bias):
    from concourse.bass_utils import run_bass_kernel_spmd

    nc = _program()
    in_maps = _shards(x, weight, bias)
    res = run_bass_kernel_spmd(nc, in_maps, core_ids=list(range(NCORES)))
    return _gather(res.results)
